# revision 71
# baseline (speedup 1.0000x reference)
"""Transformer-XL relative attention (B=2, L=2048, D=1024, H=16) on 8 TRN2
NeuronCores.

Sharding: data-parallel over batch x tensor-parallel over heads.  Core
c = 4*b + g handles batch b, head group g (4 heads).  Wq/Wk/Wv are
column-sharded, Wo row-sharded; each core emits a partial [2048,1024]
bf16 output which the host sums per batch (+bo).

Numerics: projections and the AV/output matmuls run in bf16 (f32 PSUM);
only the content and rel score matmuls run in fp8e4m3 with
MatmulPerfMode.DoubleRow (contraction 2x32 per head, 0.5 cycles/row).
The q/k/pe projection weights are column-permuted host-side so each
PSUM eviction lands lane-aligned in the DR-32 layout
  tile[32*h + r, i*2048 + n] = proj[n, 64*h + 32*i + r].

Rel-shift: R_s for query block bi (rows ri..ri+127) is stored
RIGHT-ALIGNED: rs[rr, y] = Q2[ri+rr] . peh[y] for y in [2048-Wb, 2048),
so the rel matmul's peh columns align identically for every block and
the causal pad (-1e9) at columns [2048, 2176) is written once per
buffer.  The staged chunk needed by the scores at columns [0, Wb) is
staged[rr, c] = rs[rr, 2047 - ri + c - rr] -- an anti-diagonal flat
access pattern (offset 2047-ri, ap=[[PITCH-1,128],[1,Wb]]) which only
DMA engines can execute (SBUF->SBUF).  The anti-diagonal read maps the
strict upper triangle exactly onto the -1e9 pad, so exp() yields the
causal zeros with no masking pass.

Schedule: a software pipeline over 64 (head, query-block) steps.  Each
step preps block t+A (rel matmuls, rs evict, staging DMA) and consumes
block t (ident-add + content matmul per 512-chunk, exp, then PE
transposes lagged tr_lag chunks behind the exp stream, DVE-evicted
into the pt strip).  The k/v projections and vh assembly are deferred
units interleaved into the early steps; AV(h,g) only needs key strips
ct <= 4g+3 so ascending g keeps them ahead of the consumers.  The
output projection for each query group is spread one rt per step.
"""

import numpy as np
import ml_dtypes
import concourse.bass as bass
import concourse.mybir as mybir
import concourse.tile as tile
from concourse import bacc
from concourse import bass_utils
from concourse.bass_interp import get_hw_module
from concourse.masks import make_identity

BF = mybir.dt.bfloat16
F32 = mybir.dt.float32
FP8 = mybir.dt.float8e4
DR = mybir.MatmulPerfMode.DoubleRow
EXP = mybir.ActivationFunctionType.Exp
IDENT = mybir.ActivationFunctionType.Identity
MULT = mybir.AluOpType.mult
ADD = mybir.AluOpType.add

L = 2048          # sequence length
RS_W = 2176       # R_s tile width (2048 + 128 pad); anti-diag pitch = RS_W-1

PROFILE = False       # set by test harness to capture a trace
LAST_RESULTS = None   # BassKernelResults of the last run (for profiling)

# tuning knobs (read at build time)
CFG = {
    "diag_eng": "sync",    # engine issuing anti-diagonal staging DMAs
    "load_eng": "sync",    # engine issuing input/weight loads
    "rs_bufs": 4,
    "stg_bufs": 5,
    "stg_split": 4,
    "pn_bufs": 6,
    "pt_bufs": 2,
    "pipe_ahead": 4,
    "tr_lag": 2,           # chunk-lag of transposes behind exp
    "psS_bufs": 3,
    "psR_bufs": 2,
    "psT_bufs": 2,
    "psA_bufs": 1,
    "osb_bufs": 4,
    "rec64_bufs": 1,
    "r_act_frac": (1, 2),  # rs-evict on act for num of every den
    "io_bufs": 3,
    "v_per_step": 3,
    "g_order": (0, 1, 2, 3),
}


def emit_core(nc, ins, out):
    """ins: dict name->AP (DRAM), out: AP (DRAM [2048,1024] bf16)."""
    with tile.TileContext(nc) as tc:
        deng = getattr(nc, CFG["diag_eng"])
        leng = getattr(nc, CFG["load_eng"])
        with (
            tc.tile_pool(name="wgt", bufs=1) as wp,
            tc.tile_pool(name="io", bufs=CFG["io_bufs"]) as iop,
            tc.tile_pool(name="per", bufs=1) as per,
            tc.tile_pool(name="work", bufs=3) as wk,
            tc.tile_pool(name="pt", bufs=CFG["pt_bufs"]) as ptp,
            tc.tile_pool(name="psS", bufs=CFG["psS_bufs"], space="PSUM") as psS,
            tc.tile_pool(name="psR", bufs=CFG["psR_bufs"], space="PSUM") as psR,
            tc.tile_pool(name="psA", bufs=CFG["psA_bufs"], space="PSUM") as psA,
            tc.tile_pool(name="psT", bufs=CFG["psT_bufs"], space="PSUM") as psT,
        ):
            # ---------------- persistent tiles ----------------
            q1t = per.tile([128, 2 * L], FP8, tag="q1t")
            q2t = per.tile([128, 2 * L], FP8, tag="q2t")
            kt = per.tile([128, 2 * L], FP8, tag="kt")
            peht = per.tile([128, 2 * L], FP8, tag="peht")
            at = per.tile([128, 2 * L], BF, tag="at")
            vh = per.tile([128, 16 * 260], BF, tag="vh")
            vht = per.tile([128, 2 * L], BF, tag="vht")
            b1s = per.tile([128, 2], F32, tag="b1s")
            b2s = per.tile([128, 2], F32, tag="b2s")
            bks = per.tile([128, 2], F32, tag="bks")
            bvs = per.tile([128, 2], F32, tag="bvs")

            # right-aligned R_s buffers with persistent -1e9 causal pad
            rs_bufs = [per.tile([128, RS_W], BF, tag=f"rs{j}", name=f"rs{j}")
                       for j in range(CFG["rs_bufs"])]
            for rst in rs_bufs:
                nc.gpsimd.memset(rst[:, 2048:RS_W], -1e9)

            ident = per.tile([128, 128], BF, tag="ident")
            make_identity(nc, ident[:])
            dbs = per.tile([128, 2], F32, tag="dbs")
            ones_cols = vh[:].rearrange("p (ct c) -> p ct c", c=260)
            for h in range(4):
                nc.gpsimd.memset(ones_cols[:, :, 65 * h + 64], 1.0)

            def load_w(name, dt=BF):
                t = wp.tile([128, 2048], dt, tag=name, name=f"w_{name}")
                leng.dma_start(
                    t[:], ins[name].rearrange("(a p) e -> p a e", p=128)
                )
                return t

            # ------------- projections (bf16) -------------
            def evict_q(et, rc, ps):
                cs = 2048 * et + 512 * rc
                nc.scalar.activation(q1t[:, cs:cs + 512], ps[:], IDENT,
                                     bias=b1s[:, et:et + 1], scale=1.0)
                nc.scalar.activation(q2t[:, cs:cs + 512], ps[:], IDENT,
                                     bias=b2s[:, et:et + 1], scale=1.0)

            def evict_k(et, rc, ps):
                cs = 2048 * et + 512 * rc
                nc.scalar.activation(kt[:, cs:cs + 512], ps[:], IDENT,
                                     bias=bks[:, et:et + 1], scale=1.0)

            def evict_v(et, rc, ps):
                cs = 2048 * et + 512 * rc
                nc.scalar.activation(vht[:, cs:cs + 512], ps[:], IDENT,
                                     bias=bvs[:, et:et + 1], scale=1.0)

            def evict_pe(et, rc, ps):
                cs = 2048 * et + 512 * rc
                nc.vector.tensor_copy(peht[:, cs:cs + 512], ps[:])

            def proj_group(w_s, xts, et, rc, evict, xname):
                ps = psS.tile([128, 512], F32, tag="S",
                              name=f"ps_{xname}_{et}_{rc}")
                xh = xts[rc // 2]
                no = 512 * (rc % 2)
                for kc in range(8):
                    nc.tensor.matmul(
                        ps[:],
                        w_s[:, 256 * kc + 128 * et: 256 * kc + 128 * et + 128],
                        xh[:, 1024 * kc + no: 1024 * kc + no + 512],
                        start=(kc == 0),
                        stop=(kc == 7),
                    )
                evict(et, rc, ps)

            def load_x(xname, dt=BF):
                xts = []
                for nh in range(2):
                    xt = iop.tile([128, 8 * 1024], dt, tag="inT",
                                  name=f"in_{xname}_{nh}")
                    leng.dma_start(
                        xt[:].rearrange("p (a n) -> p a n", a=8),
                        ins[xname][:, 1024 * nh: 1024 * nh + 1024]
                        .rearrange("(a p) n -> p a n", p=128),
                    )
                    xts.append(xt)
                return xts

            def project(xname, w_s, evict):
                xt = load_x(xname)
                for rc in range(4):
                    for et in range(2):
                        proj_group(w_s, xt, et, rc, evict, xname)

            # wq in halves: the first four kc matmuls only need a-blocks
            # 0-3, so the first projection group starts ~0.7us earlier
            wq_s = wp.tile([128, 2048], BF, tag="wq", name="w_wq")
            leng.dma_start(
                wq_s[:, 0:1024],
                ins["wq"][0:512].rearrange("(a p) e -> p a e", p=128))
            # qT loaded in quarters so the first projection group starts
            # as early as possible
            xqs = []
            for nq in range(4):
                xt = iop.tile([128, 8 * 512], BF, tag="inT",
                              name=f"in_qT_{nq}")
                leng.dma_start(
                    xt[:].rearrange("p (a n) -> p a n", a=8),
                    ins["qT"][:, 512 * nq: 512 * nq + 512]
                    .rearrange("(a p) n -> p a n", p=128),
                )
                xqs.append(xt)
            leng.dma_start(
                wq_s[:, 1024:2048],
                ins["wq"][512:1024].rearrange("(a p) e -> p a e", p=128))
            nc.sync.dma_start(b1s[:], ins["b1"])
            nc.sync.dma_start(b2s[:], ins["b2"])
            nc.sync.dma_start(bks[:], ins["bk"])
            nc.sync.dma_start(bvs[:], ins["bv"])
            nc.vector.tensor_tensor(out=dbs[:], in0=b2s[:], in1=b1s[:],
                                    op=mybir.AluOpType.subtract)
            for rc in range(4):
                for et in range(2):
                    ps = psS.tile([128, 512], F32, tag="S",
                                  name=f"ps_qT_{et}_{rc}")
                    for kc in range(8):
                        nc.tensor.matmul(
                            ps[:],
                            wq_s[:, 256 * kc + 128 * et:
                                 256 * kc + 128 * et + 128],
                            xqs[rc][:, 512 * kc: 512 * kc + 512],
                            start=(kc == 0),
                            stop=(kc == 7),
                        )
                    evict_q(et, rc, ps)
            rk_s = load_w("rk", FP8)
            pxts = load_x("peT", FP8)
            rk_r = rk_s[:].rearrange("p (a e) -> p a e", a=8)
            for rc in range(4):
                xh = pxts[rc // 2]
                x_r = xh[:].rearrange("p (a n) -> p a n", a=8)
                no = 512 * (rc % 2)
                for et in range(2):
                    ps = psS.tile([128, 512], F32, tag="S",
                                  name=f"ps_peT_{et}_{rc}")
                    for kc in range(4):
                        nc.tensor.matmul(
                            ps[:],
                            rk_r[:, 2 * kc: 2 * kc + 2,
                                 128 * et: 128 * et + 128],
                            x_r[:, 2 * kc: 2 * kc + 2, no: no + 512],
                            start=(kc == 0),
                            stop=(kc == 3),
                            perf_mode=DR,
                        )
                    evict_pe(et, rc, ps)
            wk_s = load_w("wk")
            kxt = load_x("kT")
            wv_s = load_w("wv")
            vxt = load_x("vT")
            wo_s = load_w("wo")

            # deferred v-projection + vh assembly units (interleaved below)
            def v_group(et, rc):
                proj_group(wv_s, vxt, et, rc, evict_v, "vT")

            def vh_asm(ct):
                # PE-transpose one [128,128] tile per (ct, pair); evict the
                # two heads' column halves into their vh blocks.
                for p in range(2):
                    scr = psT.tile([128, 512], BF, tag="T",
                                   name=f"scr_{ct}_{p}")
                    nc.tensor.transpose(
                        scr[:, 0:128],
                        vht[:, 2048 * p + 128 * ct: 2048 * p + 128 * ct + 128],
                        ident[:],
                    )
                    for hh in range(2):
                        h4 = 2 * p + hh
                        nc.vector.tensor_copy(
                            vh[:, 260 * ct + 65 * h4: 260 * ct + 65 * h4 + 64],
                            scr[:, 64 * hh: 64 * hh + 64],
                        )

            vunits = [lambda et=et, rc=rc: proj_group(wk_s, kxt, et, rc,
                                                      evict_k, "kT")
                      for rc in range(4) for et in range(2)]
            for rc in range(4):
                vunits.append(lambda rc=rc: v_group(0, rc))
                vunits.append(lambda rc=rc: v_group(1, rc))
                for ct in range(4 * rc, 4 * rc + 4):
                    vunits.append(lambda ct=ct: vh_asm(ct))

            # ------------- output projection -------------
            def outproj_rt(rt):
                osb = wk.tile([128, 1024], BF, tag="osb",
                              bufs=CFG["osb_bufs"], name=f"osb_{rt}")
                for n in range(2):
                    op_ = psS.tile([128, 512], F32, tag="S",
                                   name=f"op_{rt}_{n}")
                    for hc in range(2):
                        nc.tensor.matmul(
                            op_[:],
                            at[:, 2048 * hc + 128 * rt:
                               2048 * hc + 128 * rt + 128],
                            wo_s[:, 1024 * hc + 512 * n:
                                 1024 * hc + 512 * n + 512],
                            start=(hc == 0), stop=(hc == 1),
                        )
                    if n == 0:
                        nc.scalar.copy(osb[:, 0:512], op_[:])
                    else:
                        nc.vector.tensor_copy(osb[:, 512:1024], op_[:])
                nc.sync.dma_start(out[128 * rt: 128 * rt + 128, :], osb[:])

            # ---------------- attention pipeline ----------------
            q1_r = q1t[:].rearrange("p (i n) -> p i n", i=2)
            q2_r = q2t[:].rearrange("p (i n) -> p i n", i=2)
            kt_r = kt[:].rearrange("p (i n) -> p i n", i=2)
            pe_r = peht[:].rearrange("p (i n) -> p i n", i=2)

            # g ascending: AV(h, g) only needs vh key strips ct <= 4g+3, so
            # the interleaved v units stay ahead of the AV consumers.
            hg_order = [(h, g) for g in CFG["g_order"] for h in range(4)]
            blocks = [(h, g, bi) for h, g in hg_order
                      for bi in range(4 * g, 4 * g + 4)]
            pts = {}
            counters = {"rs": 0, "pair": 0, "xeng": 0}

            def emit_prep(t):
                """rel matmuls + rs evict + anti-diag staging DMA."""
                h, g, bi = blocks[t]
                p0, p1 = 32 * h, 32 * h + 32
                ri = 128 * bi
                Wb = ri + 128
                nch = (Wb + 511) // 512
                y0 = 2048 - Wb     # right-aligned R_s start column
                rs = rs_bufs[t % CFG["rs_bufs"]]
                for jc in range(nch):
                    w = min(512, Wb - 512 * jc)
                    rp = psR.tile([128, 512], F32, tag="R",
                                  name=f"rp_h{h}_b{bi}_{jc}")
                    nc.tensor.matmul(
                        rp[:, :w],
                        q2_r[p0:p1, :, ri:ri + 128],
                        pe_r[p0:p1, :, y0 + 512 * jc: y0 + 512 * jc + w],
                        start=True, stop=True,
                        perf_mode=DR,
                        tile_position=(p0, 0),
                    )
                    dst = rs[:, y0 + 512 * jc: y0 + 512 * jc + w]
                    num, den = CFG["r_act_frac"]
                    if counters["rs"] % den < num:
                        nc.scalar.copy(dst, rp[:, :w])
                    else:
                        nc.vector.tensor_copy(dst, rp[:, :w])
                    counters["rs"] += 1
                staged = wk.tile([128, 2048], BF, tag="stg",
                                 bufs=CFG["stg_bufs"],
                                 name=f"stg_h{h}_b{bi}")
                nsp = CFG["stg_split"] if Wb >= 1024 else 1
                for sj in range(nsp):
                    c0 = Wb * sj // nsp
                    c1 = Wb * (sj + 1) // nsp
                    diag = bass.AP(
                        tensor=rs.tensor,
                        offset=rs.offset + y0 + 127 + c0,
                        ap=[[RS_W - 1, 128], [1, c1 - c0]],
                    )
                    deng.dma_start(staged[:, c0:c1], diag)
                return staged

            def emit_consume(t, staged):
                """ident-add + content (fp8 DR), exp, lagged PE transpose
                + pt evict per 512-chunk."""
                h, g, bi = blocks[t]
                p0, p1 = 32 * h, 32 * h + 32
                ri = 128 * bi
                Wb = ri + 128
                nch = (Wb + 511) // 512
                if (h, g) not in pts:
                    pts[(h, g)] = ptp.tile(
                        [128, 512 * (4 * g + 4)], BF, tag="pt",
                        bufs=CFG["pt_bufs"], name=f"pt_h{h}_g{g}")
                pt = pts[(h, g)]
                strip = 512 * (4 * g + 4)

                def emit_pt(jc, pn, wc):
                    c0 = 512 * jc
                    tp_ = psT.tile([128, 512], BF, tag="T",
                                   name=f"tp_h{h}_b{bi}_{jc}")
                    for s in range(wc // 128):
                        nc.tensor.transpose(
                            tp_[:, 128 * s: 128 * s + 128],
                            pn[:, 128 * s: 128 * s + 128],
                            ident[:],
                        )
                    dst = bass.AP(
                        tensor=pt.tensor,
                        offset=pt.offset + 512 * (c0 // 128) + 128 * (bi % 4),
                        ap=[[strip, 128], [512, wc // 128], [1, 128]],
                    )
                    nc.vector.tensor_copy(dst, tp_[:, :wc])

                lag = CFG["tr_lag"]
                pns = {}
                for jc in range(nch):
                    c0 = 512 * jc
                    wc = min(512, Wb - c0)
                    sp = psS.tile([128, 512], F32, tag="S",
                                  name=f"sp_h{h}_b{bi}_{jc}")
                    nc.tensor.matmul(
                        sp[:, :wc],
                        ident[:],
                        staged[:, c0:c0 + wc],
                        start=True, stop=False,
                    )
                    nc.tensor.matmul(
                        sp[:, :wc],
                        q1_r[p0:p1, :, ri:ri + 128],
                        kt_r[p0:p1, :, c0:c0 + wc],
                        start=False, stop=True,
                        perf_mode=DR,
                        tile_position=(p0, 0),
                    )
                    pn = wk.tile([128, 512], BF, tag="pn",
                                 bufs=CFG["pn_bufs"],
                                 name=f"pn_h{h}_b{bi}_{jc}")
                    nc.scalar.activation(pn[:, :wc], sp[:, :wc], EXP,
                                         scale=0.125)
                    pns[jc] = (pn, wc)
                    if jc >= lag:
                        emit_pt(jc - lag, *pns.pop(jc - lag))
                for jc in range(max(0, nch - lag), nch):
                    emit_pt(jc, *pns.pop(jc))

            def emit_av(h, g):
                """AV matmul + softmax normalization; row 64 = denominators."""
                pt = pts.pop((h, g))
                av = psA.tile([65, 512], F32, tag="A", name=f"av_h{h}_g{g}")
                for ci in range(4 * g + 4):
                    o = max(0, 128 * ci - 512 * g)
                    nc.tensor.matmul(
                        av[:, o:512],
                        vh[:, 260 * ci + 65 * h: 260 * ci + 65 * h + 65],
                        pt[:, 512 * ci + o: 512 * ci + 512],
                        start=(ci == 0), stop=(ci == 4 * g + 3),
                    )
                rec = wk.tile([1, 512], F32, tag="rec", name=f"rec_h{h}_g{g}")
                nc.vector.reciprocal(rec[:], av[64:65, :])
                rec64 = wk.tile([64, 512], F32, tag="rec64",
                                bufs=CFG["rec64_bufs"],
                                name=f"rec64_h{h}_g{g}")
                nc.gpsimd.partition_broadcast(rec64[:], rec[:])
                r0 = 64 * (h % 2)
                et = h // 2
                nc.vector.tensor_tensor(
                    out=at[r0:r0 + 64, 2048 * et + 512 * g:
                           2048 * et + 512 * g + 512],
                    in0=av[0:64, :],
                    in1=rec64[:],
                    op=MULT,
                )

            A = CFG["pipe_ahead"]
            staged_q = {}
            vq = list(vunits)
            pq = []
            for t in range(len(blocks) + A):
                if t < len(blocks):
                    staged_q[t] = emit_prep(t)
                for _ in range(CFG["v_per_step"]):
                    if vq:
                        vq.pop(0)()
                if pq:
                    pq.pop(0)()
                tcons = t - A
                if tcons >= 0:
                    h, g, bi = blocks[tcons]
                    emit_consume(tcons, staged_q.pop(tcons))
                    if bi == 4 * g + 3:
                        emit_av(h, g)
                        if h == 3:
                            pq.extend(
                                lambda rt=rt: outproj_rt(rt)
                                for rt in range(4 * g, 4 * g + 4))
            for u in pq:
                u()
    return nc


# ---------------- host side ----------------

FP8NP = ml_dtypes.float8_e4m3


def _bf16(x):
    return np.ascontiguousarray(x).astype(ml_dtypes.bfloat16)


def _col2d(vec256):
    """[256] f32 -> [128, 2] with v2d[p, a] = vec[128a + p]."""
    return np.ascontiguousarray(
        np.asarray(vec256, np.float32).reshape(2, 128).T)


# DR-32 column permutation: new col position (et, q) holds original
# d = 64*(q//32) + 32*et + q%32 of the 256-slice.
_DMAP = np.empty(256, np.int64)
for _et in range(2):
    for _q in range(128):
        _DMAP[128 * _et + _q] = 64 * (_q // 32) + 32 * _et + (_q % 32)


def core_inputs(q_b, k_b, v_b, pos_enc, Wq, bq, Wk, bk, Wv, bv, Wo,
                r_w_bias, r_r_bias, r_kernel, g):
    sl = slice(256 * g, 256 * g + 256)
    rk_cat = np.concatenate([r_kernel[4 * g + i] for i in range(4)], axis=1)
    b1 = (np.asarray(bq)[sl]
          + np.asarray(r_w_bias)[4 * g:4 * g + 4].reshape(256))
    b2 = (np.asarray(bq)[sl]
          + np.asarray(r_r_bias)[4 * g:4 * g + 4].reshape(256))
    return {
        "qT": _bf16(q_b.T),
        "kT": _bf16(k_b.T),
        "vT": _bf16(v_b.T),
        "peT": np.ascontiguousarray(
            pos_enc[1:2049].T / 16.0).astype(FP8NP),
        "wq": _bf16(np.asarray(Wq)[:, sl][:, _DMAP]),
        "wk": _bf16(np.asarray(Wk)[:, sl][:, _DMAP]),
        "wv": _bf16(np.asarray(Wv)[:, sl]),
        "rk": np.ascontiguousarray(
            16.0 * rk_cat[:, _DMAP]).astype(FP8NP),
        "wo": _bf16(np.asarray(Wo)[sl, :]),
        "b1": _col2d(b1[_DMAP]),
        "b2": _col2d(b2[_DMAP]),
        "bk": _col2d(np.asarray(bk)[sl][_DMAP]),
        "bv": _col2d(np.asarray(bv)[sl]),
    }


_SHAPES = {
    "qT": ([1024, 2048], BF), "kT": ([1024, 2048], BF),
    "vT": ([1024, 2048], BF), "peT": ([1024, 2048], FP8),
    "wq": ([1024, 256], BF), "wk": ([1024, 256], BF),
    "wv": ([1024, 256], BF),
    "rk": ([1024, 256], FP8), "wo": ([256, 1024], BF),
    "b1": ([128, 2], F32), "b2": ([128, 2], F32),
    "bk": ([128, 2], F32), "bv": ([128, 2], F32),
}

_NC_CACHE = {}


def _build():
    key = tuple(sorted((k, tuple(v) if isinstance(v, list) else v)
                       for k, v in CFG.items()))
    if key in _NC_CACHE:
        return _NC_CACHE[key]
    nc = bacc.Bacc("TRN2", target_bir_lowering=False, debug=False,
                   enable_asserts=False)
    ins = {name: nc.dram_tensor(name, shape, dt, kind="ExternalInput").ap()
           for name, (shape, dt) in _SHAPES.items()}
    out = nc.dram_tensor("out", [2048, 1024], BF, kind="ExternalOutput").ap()
    emit_core(nc, ins, out)
    nc.compile()
    nc.m = get_hw_module(nc.m)
    _NC_CACHE[key] = nc
    return nc


def kernel(**inputs):
    global LAST_RESULTS
    inp = {k: np.asarray(v) for k, v in inputs.items()}
    nc = _build()
    in_maps = []
    for c in range(8):
        b, g = c // 4, c % 4
        in_maps.append(core_inputs(
            inp["q"][b], inp["k"][b], inp["v"][b], inp["pos_enc"],
            inp["Wq"], inp["bq"], inp["Wk"], inp["bk"], inp["Wv"], inp["bv"],
            inp["Wo"], inp["r_w_bias"], inp["r_r_bias"], inp["r_kernel"], g))
    res = bass_utils.run_bass_kernel_spmd(
        nc, in_maps, core_ids=list(range(8)), trace=PROFILE)
    LAST_RESULTS = res
    out = np.zeros((2, 2048, 1024), np.float32)
    for c in range(8):
        b = c // 4
        out[b] += np.asarray(res.results[c]["out"], np.float32)
    out += np.asarray(inp["bo"], np.float32)[None, None, :]
    return out


# revision 72
# speedup vs baseline: 1.0004x; 1.0004x over previous
"""Transformer-XL relative attention (B=2, L=2048, D=1024, H=16) on 8 TRN2
NeuronCores.

Sharding: data-parallel over batch x tensor-parallel over heads.  Core
c = 4*b + g handles batch b, head group g (4 heads).  Wq/Wk/Wv are
column-sharded, Wo row-sharded; each core emits a partial [2048,1024]
bf16 output which the host sums per batch (+bo).

Numerics: projections and the AV/output matmuls run in bf16 (f32 PSUM);
only the content and rel score matmuls run in fp8e4m3 with
MatmulPerfMode.DoubleRow (contraction 2x32 per head, 0.5 cycles/row).
The q/k/pe projection weights are column-permuted host-side so each
PSUM eviction lands lane-aligned in the DR-32 layout
  tile[32*h + r, i*2048 + n] = proj[n, 64*h + 32*i + r].

Rel-shift: R_s for query block bi (rows ri..ri+127) is stored
RIGHT-ALIGNED: rs[rr, y] = Q2[ri+rr] . peh[y] for y in [2048-Wb, 2048),
so the rel matmul's peh columns align identically for every block and
the causal pad (-1e9) at columns [2048, 2176) is written once per
buffer.  The staged chunk needed by the scores at columns [0, Wb) is
staged[rr, c] = rs[rr, 2047 - ri + c - rr] -- an anti-diagonal flat
access pattern (offset 2047-ri, ap=[[PITCH-1,128],[1,Wb]]) which only
DMA engines can execute (SBUF->SBUF).  The anti-diagonal read maps the
strict upper triangle exactly onto the -1e9 pad, so exp() yields the
causal zeros with no masking pass.

Schedule: a software pipeline over 64 (head, query-block) steps.  Each
step preps block t+A (rel matmuls, rs evict, staging DMA) and consumes
block t (ident-add + content matmul per 512-chunk, exp, then PE
transposes lagged tr_lag chunks behind the exp stream, DVE-evicted
into the pt strip).  The k/v projections and vh assembly are deferred
units interleaved into the early steps; AV(h,g) only needs key strips
ct <= 4g+3 so ascending g keeps them ahead of the consumers.  The
output projection for each query group is spread one rt per step.
"""

import numpy as np
import ml_dtypes
import concourse.bass as bass
import concourse.mybir as mybir
import concourse.tile as tile
from concourse import bacc
from concourse import bass_utils
from concourse.bass_interp import get_hw_module
from concourse.masks import make_identity

BF = mybir.dt.bfloat16
F32 = mybir.dt.float32
FP8 = mybir.dt.float8e4
DR = mybir.MatmulPerfMode.DoubleRow
EXP = mybir.ActivationFunctionType.Exp
IDENT = mybir.ActivationFunctionType.Identity
MULT = mybir.AluOpType.mult
ADD = mybir.AluOpType.add

L = 2048          # sequence length
RS_W = 2176       # R_s tile width (2048 + 128 pad); anti-diag pitch = RS_W-1

PROFILE = False       # set by test harness to capture a trace
LAST_RESULTS = None   # BassKernelResults of the last run (for profiling)

# tuning knobs (read at build time)
CFG = {
    "diag_eng": "sync",    # engine issuing anti-diagonal staging DMAs
    "load_eng": "sync",    # engine issuing input/weight loads
    "rs_bufs": 4,
    "stg_bufs": 6,
    "stg_split": 4,
    "pn_bufs": 6,
    "pt_bufs": 2,
    "pipe_ahead": 4,
    "tr_lag": 2,           # chunk-lag of transposes behind exp
    "psS_bufs": 3,
    "psR_bufs": 2,
    "psT_bufs": 2,
    "psA_bufs": 1,
    "osb_bufs": 6,
    "rec64_bufs": 1,
    "r_act_frac": (1, 2),  # rs-evict on act for num of every den
    "io_bufs": 3,
    "v_per_step": 3,
    "g_order": (0, 1, 2, 3),
}


def emit_core(nc, ins, out):
    """ins: dict name->AP (DRAM), out: AP (DRAM [2048,1024] bf16)."""
    with tile.TileContext(nc) as tc:
        deng = getattr(nc, CFG["diag_eng"])
        leng = getattr(nc, CFG["load_eng"])
        with (
            tc.tile_pool(name="wgt", bufs=1) as wp,
            tc.tile_pool(name="io", bufs=CFG["io_bufs"]) as iop,
            tc.tile_pool(name="per", bufs=1) as per,
            tc.tile_pool(name="work", bufs=3) as wk,
            tc.tile_pool(name="pt", bufs=CFG["pt_bufs"]) as ptp,
            tc.tile_pool(name="psS", bufs=CFG["psS_bufs"], space="PSUM") as psS,
            tc.tile_pool(name="psR", bufs=CFG["psR_bufs"], space="PSUM") as psR,
            tc.tile_pool(name="psA", bufs=CFG["psA_bufs"], space="PSUM") as psA,
            tc.tile_pool(name="psT", bufs=CFG["psT_bufs"], space="PSUM") as psT,
        ):
            # ---------------- persistent tiles ----------------
            q1t = per.tile([128, 2 * L], FP8, tag="q1t")
            q2t = per.tile([128, 2 * L], FP8, tag="q2t")
            kt = per.tile([128, 2 * L], FP8, tag="kt")
            peht = per.tile([128, 2 * L], FP8, tag="peht")
            at = per.tile([128, 2 * L], BF, tag="at")
            vh = per.tile([128, 16 * 260], BF, tag="vh")
            vht = per.tile([128, 2 * L], BF, tag="vht")
            b1s = per.tile([128, 2], F32, tag="b1s")
            b2s = per.tile([128, 2], F32, tag="b2s")
            bks = per.tile([128, 2], F32, tag="bks")
            bvs = per.tile([128, 2], F32, tag="bvs")

            # right-aligned R_s buffers with persistent -1e9 causal pad
            rs_bufs = [per.tile([128, RS_W], BF, tag=f"rs{j}", name=f"rs{j}")
                       for j in range(CFG["rs_bufs"])]
            for rst in rs_bufs:
                nc.gpsimd.memset(rst[:, 2048:RS_W], -1e9)

            ident = per.tile([128, 128], BF, tag="ident")
            make_identity(nc, ident[:])
            dbs = per.tile([128, 2], F32, tag="dbs")
            ones_cols = vh[:].rearrange("p (ct c) -> p ct c", c=260)
            for h in range(4):
                nc.gpsimd.memset(ones_cols[:, :, 65 * h + 64], 1.0)

            def load_w(name, dt=BF):
                t = wp.tile([128, 2048], dt, tag=name, name=f"w_{name}")
                leng.dma_start(
                    t[:], ins[name].rearrange("(a p) e -> p a e", p=128)
                )
                return t

            # ------------- projections (bf16) -------------
            def evict_q(et, rc, ps):
                cs = 2048 * et + 512 * rc
                nc.scalar.activation(q1t[:, cs:cs + 512], ps[:], IDENT,
                                     bias=b1s[:, et:et + 1], scale=1.0)
                nc.scalar.activation(q2t[:, cs:cs + 512], ps[:], IDENT,
                                     bias=b2s[:, et:et + 1], scale=1.0)

            def evict_k(et, rc, ps):
                cs = 2048 * et + 512 * rc
                nc.scalar.activation(kt[:, cs:cs + 512], ps[:], IDENT,
                                     bias=bks[:, et:et + 1], scale=1.0)

            def evict_v(et, rc, ps):
                cs = 2048 * et + 512 * rc
                nc.scalar.activation(vht[:, cs:cs + 512], ps[:], IDENT,
                                     bias=bvs[:, et:et + 1], scale=1.0)

            def evict_pe(et, rc, ps):
                cs = 2048 * et + 512 * rc
                nc.vector.tensor_copy(peht[:, cs:cs + 512], ps[:])

            def proj_group(w_s, xts, et, rc, evict, xname):
                ps = psS.tile([128, 512], F32, tag="S",
                              name=f"ps_{xname}_{et}_{rc}")
                xh = xts[rc // 2]
                no = 512 * (rc % 2)
                for kc in range(8):
                    nc.tensor.matmul(
                        ps[:],
                        w_s[:, 256 * kc + 128 * et: 256 * kc + 128 * et + 128],
                        xh[:, 1024 * kc + no: 1024 * kc + no + 512],
                        start=(kc == 0),
                        stop=(kc == 7),
                    )
                evict(et, rc, ps)

            def load_x(xname, dt=BF):
                xts = []
                for nh in range(2):
                    xt = iop.tile([128, 8 * 1024], dt, tag="inT",
                                  name=f"in_{xname}_{nh}")
                    leng.dma_start(
                        xt[:].rearrange("p (a n) -> p a n", a=8),
                        ins[xname][:, 1024 * nh: 1024 * nh + 1024]
                        .rearrange("(a p) n -> p a n", p=128),
                    )
                    xts.append(xt)
                return xts

            def project(xname, w_s, evict):
                xt = load_x(xname)
                for rc in range(4):
                    for et in range(2):
                        proj_group(w_s, xt, et, rc, evict, xname)

            # wq in halves: the first four kc matmuls only need a-blocks
            # 0-3, so the first projection group starts ~0.7us earlier
            wq_s = wp.tile([128, 2048], BF, tag="wq", name="w_wq")
            leng.dma_start(
                wq_s[:, 0:1024],
                ins["wq"][0:512].rearrange("(a p) e -> p a e", p=128))
            # qT loaded in quarters so the first projection group starts
            # as early as possible
            xqs = []
            for nq in range(4):
                xt = iop.tile([128, 8 * 512], BF, tag="inT",
                              name=f"in_qT_{nq}")
                leng.dma_start(
                    xt[:].rearrange("p (a n) -> p a n", a=8),
                    ins["qT"][:, 512 * nq: 512 * nq + 512]
                    .rearrange("(a p) n -> p a n", p=128),
                )
                xqs.append(xt)
            leng.dma_start(
                wq_s[:, 1024:2048],
                ins["wq"][512:1024].rearrange("(a p) e -> p a e", p=128))
            nc.sync.dma_start(b1s[:], ins["b1"])
            nc.sync.dma_start(b2s[:], ins["b2"])
            nc.sync.dma_start(bks[:], ins["bk"])
            nc.sync.dma_start(bvs[:], ins["bv"])
            nc.vector.tensor_tensor(out=dbs[:], in0=b2s[:], in1=b1s[:],
                                    op=mybir.AluOpType.subtract)
            for rc in range(4):
                for et in range(2):
                    ps = psS.tile([128, 512], F32, tag="S",
                                  name=f"ps_qT_{et}_{rc}")
                    for kc in range(8):
                        nc.tensor.matmul(
                            ps[:],
                            wq_s[:, 256 * kc + 128 * et:
                                 256 * kc + 128 * et + 128],
                            xqs[rc][:, 512 * kc: 512 * kc + 512],
                            start=(kc == 0),
                            stop=(kc == 7),
                        )
                    evict_q(et, rc, ps)
            rk_s = load_w("rk", FP8)
            pxts = load_x("peT", FP8)
            rk_r = rk_s[:].rearrange("p (a e) -> p a e", a=8)
            for rc in range(4):
                xh = pxts[rc // 2]
                x_r = xh[:].rearrange("p (a n) -> p a n", a=8)
                no = 512 * (rc % 2)
                for et in range(2):
                    ps = psS.tile([128, 512], F32, tag="S",
                                  name=f"ps_peT_{et}_{rc}")
                    for kc in range(4):
                        nc.tensor.matmul(
                            ps[:],
                            rk_r[:, 2 * kc: 2 * kc + 2,
                                 128 * et: 128 * et + 128],
                            x_r[:, 2 * kc: 2 * kc + 2, no: no + 512],
                            start=(kc == 0),
                            stop=(kc == 3),
                            perf_mode=DR,
                        )
                    evict_pe(et, rc, ps)
            wk_s = load_w("wk")
            kxt = load_x("kT")
            wv_s = load_w("wv")
            vxt = load_x("vT")
            wo_s = load_w("wo")

            # deferred v-projection + vh assembly units (interleaved below)
            def v_group(et, rc):
                proj_group(wv_s, vxt, et, rc, evict_v, "vT")

            def vh_asm(ct):
                # PE-transpose one [128,128] tile per (ct, pair); evict the
                # two heads' column halves into their vh blocks.
                for p in range(2):
                    scr = psT.tile([128, 512], BF, tag="T",
                                   name=f"scr_{ct}_{p}")
                    nc.tensor.transpose(
                        scr[:, 0:128],
                        vht[:, 2048 * p + 128 * ct: 2048 * p + 128 * ct + 128],
                        ident[:],
                    )
                    for hh in range(2):
                        h4 = 2 * p + hh
                        nc.vector.tensor_copy(
                            vh[:, 260 * ct + 65 * h4: 260 * ct + 65 * h4 + 64],
                            scr[:, 64 * hh: 64 * hh + 64],
                        )

            vunits = [lambda et=et, rc=rc: proj_group(wk_s, kxt, et, rc,
                                                      evict_k, "kT")
                      for rc in range(4) for et in range(2)]
            for rc in range(4):
                vunits.append(lambda rc=rc: v_group(0, rc))
                vunits.append(lambda rc=rc: v_group(1, rc))
                for ct in range(4 * rc, 4 * rc + 4):
                    vunits.append(lambda ct=ct: vh_asm(ct))

            # ------------- output projection -------------
            def outproj_rt(rt):
                osb = wk.tile([128, 1024], BF, tag="osb",
                              bufs=CFG["osb_bufs"], name=f"osb_{rt}")
                for n in range(2):
                    op_ = psS.tile([128, 512], F32, tag="S",
                                   name=f"op_{rt}_{n}")
                    for hc in range(2):
                        nc.tensor.matmul(
                            op_[:],
                            at[:, 2048 * hc + 128 * rt:
                               2048 * hc + 128 * rt + 128],
                            wo_s[:, 1024 * hc + 512 * n:
                                 1024 * hc + 512 * n + 512],
                            start=(hc == 0), stop=(hc == 1),
                        )
                    if n == 0:
                        nc.scalar.copy(osb[:, 0:512], op_[:])
                    else:
                        nc.vector.tensor_copy(osb[:, 512:1024], op_[:])
                nc.sync.dma_start(out[128 * rt: 128 * rt + 128, :], osb[:])

            # ---------------- attention pipeline ----------------
            q1_r = q1t[:].rearrange("p (i n) -> p i n", i=2)
            q2_r = q2t[:].rearrange("p (i n) -> p i n", i=2)
            kt_r = kt[:].rearrange("p (i n) -> p i n", i=2)
            pe_r = peht[:].rearrange("p (i n) -> p i n", i=2)

            # g ascending: AV(h, g) only needs vh key strips ct <= 4g+3, so
            # the interleaved v units stay ahead of the AV consumers.
            hg_order = [(h, g) for g in CFG["g_order"] for h in range(4)]
            blocks = [(h, g, bi) for h, g in hg_order
                      for bi in range(4 * g, 4 * g + 4)]
            pts = {}
            counters = {"rs": 0, "pair": 0, "xeng": 0}

            def emit_prep(t):
                """rel matmuls + rs evict + anti-diag staging DMA."""
                h, g, bi = blocks[t]
                p0, p1 = 32 * h, 32 * h + 32
                ri = 128 * bi
                Wb = ri + 128
                nch = (Wb + 511) // 512
                y0 = 2048 - Wb     # right-aligned R_s start column
                rs = rs_bufs[t % CFG["rs_bufs"]]
                for jc in range(nch):
                    w = min(512, Wb - 512 * jc)
                    rp = psR.tile([128, 512], F32, tag="R",
                                  name=f"rp_h{h}_b{bi}_{jc}")
                    nc.tensor.matmul(
                        rp[:, :w],
                        q2_r[p0:p1, :, ri:ri + 128],
                        pe_r[p0:p1, :, y0 + 512 * jc: y0 + 512 * jc + w],
                        start=True, stop=True,
                        perf_mode=DR,
                        tile_position=(p0, 0),
                    )
                    dst = rs[:, y0 + 512 * jc: y0 + 512 * jc + w]
                    num, den = CFG["r_act_frac"]
                    if counters["rs"] % den < num:
                        nc.scalar.copy(dst, rp[:, :w])
                    else:
                        nc.vector.tensor_copy(dst, rp[:, :w])
                    counters["rs"] += 1
                staged = wk.tile([128, 2048], BF, tag="stg",
                                 bufs=CFG["stg_bufs"],
                                 name=f"stg_h{h}_b{bi}")
                nsp = CFG["stg_split"] if Wb >= 1024 else 1
                for sj in range(nsp):
                    c0 = Wb * sj // nsp
                    c1 = Wb * (sj + 1) // nsp
                    diag = bass.AP(
                        tensor=rs.tensor,
                        offset=rs.offset + y0 + 127 + c0,
                        ap=[[RS_W - 1, 128], [1, c1 - c0]],
                    )
                    deng.dma_start(staged[:, c0:c1], diag)
                return staged

            def emit_consume(t, staged):
                """ident-add + content (fp8 DR), exp, lagged PE transpose
                + pt evict per 512-chunk."""
                h, g, bi = blocks[t]
                p0, p1 = 32 * h, 32 * h + 32
                ri = 128 * bi
                Wb = ri + 128
                nch = (Wb + 511) // 512
                if (h, g) not in pts:
                    pts[(h, g)] = ptp.tile(
                        [128, 512 * (4 * g + 4)], BF, tag="pt",
                        bufs=CFG["pt_bufs"], name=f"pt_h{h}_g{g}")
                pt = pts[(h, g)]
                strip = 512 * (4 * g + 4)

                def emit_pt(jc, pn, wc):
                    c0 = 512 * jc
                    tp_ = psT.tile([128, 512], BF, tag="T",
                                   name=f"tp_h{h}_b{bi}_{jc}")
                    for s in range(wc // 128):
                        nc.tensor.transpose(
                            tp_[:, 128 * s: 128 * s + 128],
                            pn[:, 128 * s: 128 * s + 128],
                            ident[:],
                        )
                    dst = bass.AP(
                        tensor=pt.tensor,
                        offset=pt.offset + 512 * (c0 // 128) + 128 * (bi % 4),
                        ap=[[strip, 128], [512, wc // 128], [1, 128]],
                    )
                    nc.vector.tensor_copy(dst, tp_[:, :wc])

                lag = CFG["tr_lag"]
                pns = {}
                for jc in range(nch):
                    c0 = 512 * jc
                    wc = min(512, Wb - c0)
                    sp = psS.tile([128, 512], F32, tag="S",
                                  name=f"sp_h{h}_b{bi}_{jc}")
                    nc.tensor.matmul(
                        sp[:, :wc],
                        ident[:],
                        staged[:, c0:c0 + wc],
                        start=True, stop=False,
                    )
                    nc.tensor.matmul(
                        sp[:, :wc],
                        q1_r[p0:p1, :, ri:ri + 128],
                        kt_r[p0:p1, :, c0:c0 + wc],
                        start=False, stop=True,
                        perf_mode=DR,
                        tile_position=(p0, 0),
                    )
                    pn = wk.tile([128, 512], BF, tag="pn",
                                 bufs=CFG["pn_bufs"],
                                 name=f"pn_h{h}_b{bi}_{jc}")
                    nc.scalar.activation(pn[:, :wc], sp[:, :wc], EXP,
                                         scale=0.125)
                    pns[jc] = (pn, wc)
                    if jc >= lag:
                        emit_pt(jc - lag, *pns.pop(jc - lag))
                for jc in range(max(0, nch - lag), nch):
                    emit_pt(jc, *pns.pop(jc))

            def emit_av(h, g):
                """AV matmul + softmax normalization; row 64 = denominators."""
                pt = pts.pop((h, g))
                av = psA.tile([65, 512], F32, tag="A", name=f"av_h{h}_g{g}")
                for ci in range(4 * g + 4):
                    o = max(0, 128 * ci - 512 * g)
                    nc.tensor.matmul(
                        av[:, o:512],
                        vh[:, 260 * ci + 65 * h: 260 * ci + 65 * h + 65],
                        pt[:, 512 * ci + o: 512 * ci + 512],
                        start=(ci == 0), stop=(ci == 4 * g + 3),
                    )
                rec = wk.tile([1, 512], F32, tag="rec", name=f"rec_h{h}_g{g}")
                nc.vector.reciprocal(rec[:], av[64:65, :])
                rec64 = wk.tile([64, 512], F32, tag="rec64",
                                bufs=CFG["rec64_bufs"],
                                name=f"rec64_h{h}_g{g}")
                nc.gpsimd.partition_broadcast(rec64[:], rec[:])
                r0 = 64 * (h % 2)
                et = h // 2
                nc.vector.tensor_tensor(
                    out=at[r0:r0 + 64, 2048 * et + 512 * g:
                           2048 * et + 512 * g + 512],
                    in0=av[0:64, :],
                    in1=rec64[:],
                    op=MULT,
                )

            A = CFG["pipe_ahead"]
            staged_q = {}
            vq = list(vunits)
            pq = []
            for t in range(len(blocks) + A):
                if t < len(blocks):
                    staged_q[t] = emit_prep(t)
                for _ in range(CFG["v_per_step"]):
                    if vq:
                        vq.pop(0)()
                if pq:
                    pq.pop(0)()
                tcons = t - A
                if tcons >= 0:
                    h, g, bi = blocks[tcons]
                    emit_consume(tcons, staged_q.pop(tcons))
                    if bi == 4 * g + 3:
                        emit_av(h, g)
                        if h == 3:
                            pq.extend(
                                lambda rt=rt: outproj_rt(rt)
                                for rt in range(4 * g, 4 * g + 4))
            for u in pq:
                u()
    return nc


# ---------------- host side ----------------

FP8NP = ml_dtypes.float8_e4m3


def _bf16(x):
    return np.ascontiguousarray(x).astype(ml_dtypes.bfloat16)


def _col2d(vec256):
    """[256] f32 -> [128, 2] with v2d[p, a] = vec[128a + p]."""
    return np.ascontiguousarray(
        np.asarray(vec256, np.float32).reshape(2, 128).T)


# DR-32 column permutation: new col position (et, q) holds original
# d = 64*(q//32) + 32*et + q%32 of the 256-slice.
_DMAP = np.empty(256, np.int64)
for _et in range(2):
    for _q in range(128):
        _DMAP[128 * _et + _q] = 64 * (_q // 32) + 32 * _et + (_q % 32)


def core_inputs(q_b, k_b, v_b, pos_enc, Wq, bq, Wk, bk, Wv, bv, Wo,
                r_w_bias, r_r_bias, r_kernel, g):
    sl = slice(256 * g, 256 * g + 256)
    rk_cat = np.concatenate([r_kernel[4 * g + i] for i in range(4)], axis=1)
    b1 = (np.asarray(bq)[sl]
          + np.asarray(r_w_bias)[4 * g:4 * g + 4].reshape(256))
    b2 = (np.asarray(bq)[sl]
          + np.asarray(r_r_bias)[4 * g:4 * g + 4].reshape(256))
    return {
        "qT": _bf16(q_b.T),
        "kT": _bf16(k_b.T),
        "vT": _bf16(v_b.T),
        "peT": np.ascontiguousarray(
            pos_enc[1:2049].T / 16.0).astype(FP8NP),
        "wq": _bf16(np.asarray(Wq)[:, sl][:, _DMAP]),
        "wk": _bf16(np.asarray(Wk)[:, sl][:, _DMAP]),
        "wv": _bf16(np.asarray(Wv)[:, sl]),
        "rk": np.ascontiguousarray(
            16.0 * rk_cat[:, _DMAP]).astype(FP8NP),
        "wo": _bf16(np.asarray(Wo)[sl, :]),
        "b1": _col2d(b1[_DMAP]),
        "b2": _col2d(b2[_DMAP]),
        "bk": _col2d(np.asarray(bk)[sl][_DMAP]),
        "bv": _col2d(np.asarray(bv)[sl]),
    }


_SHAPES = {
    "qT": ([1024, 2048], BF), "kT": ([1024, 2048], BF),
    "vT": ([1024, 2048], BF), "peT": ([1024, 2048], FP8),
    "wq": ([1024, 256], BF), "wk": ([1024, 256], BF),
    "wv": ([1024, 256], BF),
    "rk": ([1024, 256], FP8), "wo": ([256, 1024], BF),
    "b1": ([128, 2], F32), "b2": ([128, 2], F32),
    "bk": ([128, 2], F32), "bv": ([128, 2], F32),
}

_NC_CACHE = {}


def _build():
    key = tuple(sorted((k, tuple(v) if isinstance(v, list) else v)
                       for k, v in CFG.items()))
    if key in _NC_CACHE:
        return _NC_CACHE[key]
    nc = bacc.Bacc("TRN2", target_bir_lowering=False, debug=False,
                   enable_asserts=False)
    ins = {name: nc.dram_tensor(name, shape, dt, kind="ExternalInput").ap()
           for name, (shape, dt) in _SHAPES.items()}
    out = nc.dram_tensor("out", [2048, 1024], BF, kind="ExternalOutput").ap()
    emit_core(nc, ins, out)
    nc.compile()
    nc.m = get_hw_module(nc.m)
    _NC_CACHE[key] = nc
    return nc


def kernel(**inputs):
    global LAST_RESULTS
    inp = {k: np.asarray(v) for k, v in inputs.items()}
    nc = _build()
    in_maps = []
    for c in range(8):
        b, g = c // 4, c % 4
        in_maps.append(core_inputs(
            inp["q"][b], inp["k"][b], inp["v"][b], inp["pos_enc"],
            inp["Wq"], inp["bq"], inp["Wk"], inp["bk"], inp["Wv"], inp["bv"],
            inp["Wo"], inp["r_w_bias"], inp["r_r_bias"], inp["r_kernel"], g))
    res = bass_utils.run_bass_kernel_spmd(
        nc, in_maps, core_ids=list(range(8)), trace=PROFILE)
    LAST_RESULTS = res
    out = np.zeros((2, 2048, 1024), np.float32)
    for c in range(8):
        b = c // 4
        out[b] += np.asarray(res.results[c]["out"], np.float32)
    out += np.asarray(inp["bo"], np.float32)[None, None, :]
    return out
